# revision 4
# baseline (speedup 1.0000x reference)
"""Trainium2 Bass kernel for the 2-layer tanh RNN (nn_DeeperRNN), v2.

v2: interleave the two recurrences so the PE never idles.  T=512 is
processed in 4 blocks of 128 steps:

  prolog:  A1(b0), A1(b1) batched; l1-rec(b0)
  block k: A2(b k) batched + A1(b k+2) batched
           + l1-rec(b k+1) interleaved with l2-rec(b k)
  epilog:  out = h2_T @ W_h2o2.T + b_h2o2

The two recurrences have independent dependency chains, so the Tile
scheduler fills the PE gaps of one stream (its transpose+tanh latency)
with the other stream's gemv matmuls -> PE ~always busy.  Each per-step
gemv is 17 matmuls x 4 column-group streams (16 recurrent-weight chunks
+ 1 rank-1 inject of the batched pre-activation, which carries the
input term and both biases).  Only the two recurrent weight matrices
stay SBUF-resident; W_i2h1 / W_i2h2 / W_h2o2 stream through one 32KB
scratch slot.
"""

import sys
import numpy as np
import ml_dtypes

sys.path.insert(0, "/opt/trn_rl_repo")

import concourse.bass as bass  # noqa: E402
import concourse.mybir as mybir  # noqa: E402
import concourse.bacc as bacc  # noqa: E402
import concourse.tile as tile  # noqa: E402
import concourse.bass_utils as bass_utils  # noqa: E402
from contextlib import ExitStack  # noqa: E402

BF16 = mybir.dt.bfloat16
F32 = mybir.dt.float32
Tanh = mybir.ActivationFunctionType.Tanh

T, IN, H, OUT = 512, 1024, 2048, 1024
NCHUNK = H // 128  # 16
NB = 16            # blocks
BT = T // NB       # steps per block
PRIO_OFF = 1 << 20   # l1 chain outranks queued l2 work


def _host_prep(inputs):
    bf = ml_dtypes.bfloat16
    f32 = np.float32

    def perm_out_axis(a):
        # permute last axis: col (g, J, a2) = g*512 + 32*J + a2 <- row 128J + 32g + a2
        s = a.shape[:-1]
        return np.ascontiguousarray(
            a.reshape(*s, 16, 4, 32).swapaxes(-3, -2).reshape(*s, 2048)
        )

    def prep_wh(w):  # W [j, i] -> [128p, (c*4+g)*512 + J*32 + a2]
        wt = np.asarray(w, f32).T
        return np.ascontiguousarray(
            wt.reshape(16, 128, 16, 4, 32)
            .transpose(1, 0, 3, 2, 4)
            .reshape(128, 16 * 4 * 512)
            .astype(bf)
        )

    def pm(a, part=128):  # [K, N] -> [128, (K//128)*N] chunked partition-major
        k, n = a.shape
        return np.ascontiguousarray(
            a.reshape(k // part, part, n).transpose(1, 0, 2).reshape(part, -1)
        )

    x = np.asarray(inputs["word"], f32).reshape(T, IN)
    return {
        "xt": pm(np.ascontiguousarray(x.T).astype(bf)),
        "w1t": pm(perm_out_axis(np.asarray(inputs["W_i2h1"], f32).T).astype(bf)),
        "wi2t": pm(perm_out_axis(np.asarray(inputs["W_i2h2"], f32).T).astype(bf)),
        "wh1": prep_wh(inputs["W_h2h1"]),
        "wh2": prep_wh(inputs["W_h2h2"]),
        "wo2t": pm(np.asarray(inputs["W_h2o2"], f32).T.astype(bf)),
        "b1": perm_out_axis(
            np.asarray(inputs["b_i2h1"], f32) + np.asarray(inputs["b_h2h1"], f32)
        ).reshape(1, H).astype(bf),
        "b2": perm_out_axis(
            np.asarray(inputs["b_i2h2"], f32) + np.asarray(inputs["b_h2h2"], f32)
        ).reshape(1, H).astype(bf),
        "bo": np.asarray(inputs["b_h2o2"], f32).reshape(1, OUT).astype(bf),
        "ident": np.eye(128, dtype=bf),
        "ones_row": np.ones((1, 128), dtype=bf),
    }


_INPUT_SPECS = {
    "xt": ([128, (IN // 128) * T], BF16),
    "w1t": ([128, (IN // 128) * H], BF16),
    "wi2t": ([128, NCHUNK * H], BF16),
    "wh1": ([128, NCHUNK * 4 * 512], BF16),
    "wh2": ([128, NCHUNK * 4 * 512], BF16),
    "wo2t": ([128, NCHUNK * OUT], BF16),
    "b1": ([1, H], BF16),
    "b2": ([1, H], BF16),
    "bo": ([1, OUT], BF16),
    "ident": ([128, 128], BF16),
    "ones_row": ([1, 128], BF16),
}


def _build(ctx, tc, out_ap, ins):
    nc = tc.nc

    sb = lambda name, shape, dt: ctx.enter_context(nc.sbuf_tensor(name, shape, dt))

    ident = sb("identsb", [128, 128], BF16)
    nc.sync.dma_start(ident[:], ins["ident"])
    ones_row = sb("onessb", [1, 128], BF16)
    nc.sync.dma_start(ones_row[:], ins["ones_row"])
    b1_sb = sb("b1sb", [1, H], BF16)
    nc.sync.dma_start(b1_sb[:], ins["b1"])
    b2_sb = sb("b2sb", [1, H], BF16)
    nc.sync.dma_start(b2_sb[:], ins["b2"])
    bo_sb = sb("bosb", [1, OUT], BF16)
    nc.sync.dma_start(bo_sb[:], ins["bo"])

    # resident recurrent weights (64KB/partition each)
    wh1_sb = sb("wh1sb", [128, NCHUNK * 4 * 512], BF16)
    nc.sync.dma_start(wh1_sb[:], ins["wh1"])
    wh2_sb = sb("wh2sb", [128, NCHUNK * 4 * 512], BF16)
    nc.sync.dma_start(wh2_sb[:], ins["wh2"])

    # per-block pre-activations [128 t-part, 2048] bf16, rings of 2
    # (rows BT..127 stay zero; the ident-column inject must see 0 there)
    a1 = [sb(f"a1_{i}", [128, H], BF16) for i in range(2)]
    a2 = [sb(f"a2_{i}", [128, H], BF16) for i in range(2)]
    for t_ in a1 + a2:
        nc.vector.memset(t_[:], 0.0)

    # h1: ring of 3 blocks, col (i*16 + c) = h1[step i][chunk c]
    h1 = [sb(f"h1_{i}", [128, BT * 16], BF16) for i in range(3)]
    h1z = sb("h1z", [128, 16], BF16)  # h1[-1] = 0
    nc.vector.memset(h1z[:], 0.0)
    # h2: ring of 2 slots (only previous step needed) + epilog reads slot T%2
    h2 = sb("h2sb", [128, 2 * 16], BF16)
    nc.vector.memset(h2[:, 0:16], 0.0)

    ts1 = sb("ts1", [128, 512], F32)  # transpose scratch, l1 stream
    ts2 = sb("ts2", [128, 512], F32)  # transpose scratch, l2 stream

    # x blocks [128, 8*BT] bf16, ring 2 (chunk-major: col kc*BT + i)
    xt_blk = [sb(f"xtb_{i}", [128, (IN // 128) * BT], BF16) for i in range(2)]

    # streamed-weight scratch: one 32KB slot (16K bf16 cols)
    wscr = ctx.enter_context(tc.tile_pool(name="wscr", bufs=1))

    ppool = ctx.enter_context(tc.tile_pool(name="ppool", bufs=2, space="PSUM"))
    qpool = ctx.enter_context(tc.tile_pool(name="qpool", bufs=2, space="PSUM"))
    bpool = ctx.enter_context(tc.tile_pool(name="bpool", bufs=4, space="PSUM"))

    def load_xt(k):
        dst = xt_blk[k % 2]
        for kc in range(IN // 128):
            nc.sync.dma_start(
                dst[:, kc * BT:(kc + 1) * BT],
                ins["xt"].tensor.ap()[:, kc * T + k * BT: kc * T + k * BT + BT])

    def batched_a1(k):
        """A1 block k: [128 t, 2048 j] = X_blk @ W1.T + (b1+bh1)."""
        xk = xt_blk[k % 2]
        adst = a1[k % 2]
        pss = [bpool.tile([128, 512], F32, tag="pb", name=f"pa1_{k}_{ns}")
               for ns in range(4)]
        for half in range(2):  # contraction chunks 4*half .. 4*half+3
            w1 = wscr.tile([128, 8 * 1024], BF16, tag="w", name=f"w1t_{k}_{half}")
            nc.sync.dma_start(
                w1[:], ins["w1t"].tensor.ap()[:, half * 4 * H:(half + 1) * 4 * H])
            for ns in range(4):
                for kc in range(4):
                    c = half * 4 + kc
                    nc.tensor.matmul(
                        pss[ns][0:BT, :], xk[:, c * BT:(c + 1) * BT],
                        w1[:, kc * H + ns * 512: kc * H + (ns + 1) * 512],
                        start=(c == 0), stop=False)
        for ns in range(4):
            nc.tensor.matmul(
                pss[ns][0:BT, :], ones_row[:, 0:BT], b1_sb[:, ns * 512:(ns + 1) * 512],
                start=False, stop=True)
            nc.vector.tensor_copy(adst[0:BT, ns * 512:(ns + 1) * 512], pss[ns][0:BT, :])

    def batched_a2(k):
        """A2 block k = H1(block k) @ Wi2.T + (b2+bh2); wi2t streamed in
        2 contraction halves through the scratch slot."""
        hv = h1[k % 3][:].rearrange("p (t c) -> p t c", c=16)
        adst = a2[k % 2]
        pss = [bpool.tile([128, 512], F32, tag="pb", name=f"pa2_{k}_{ns}")
               for ns in range(4)]
        for q in range(4):
            wq = wscr.tile([128, 8 * 1024], BF16, tag="w", name=f"wi2_{k}_{q}")
            nc.sync.dma_start(
                wq[:], ins["wi2t"].tensor.ap()[:, q * 4 * H:(q + 1) * 4 * H])
            for ns in range(4):
                for kc in range(4):
                    c = q * 4 + kc
                    nc.tensor.matmul(
                        pss[ns][0:BT, :], hv[:, 0:BT, c:c + 1],
                        wq[:, kc * H + ns * 512: kc * H + (ns + 1) * 512],
                        start=(c == 0), stop=False)
        for ns in range(4):
            nc.tensor.matmul(
                pss[ns][0:BT, :], ones_row[:, 0:BT], b2_sb[:, ns * 512:(ns + 1) * 512],
                start=False, stop=True)
            nc.vector.tensor_copy(adst[0:BT, ns * 512:(ns + 1) * 512], pss[ns][0:BT, :])

    def l1_step(k, i):
        hk = h1[k % 3]
        if i == 0:
            hprev, pcol = (h1z, 0) if k == 0 else (h1[(k - 1) % 3], (BT - 1) * 16)
        else:
            hprev, pcol = hk, (i - 1) * 16
        ps = ppool.tile([128, 512], F32, tag="pz", name=f"pz1_{k}_{i}")
        if k == 0 and i < 2:
            nc.vector.memset(ps[:], 0.0)
        for c in range(17):
            for g in range(4):
                if c == 0:
                    lhsT = ident[:, i:i + 1]
                    rhs = a1[k % 2][:, g * 512:(g + 1) * 512]
                else:
                    cc = c - 1
                    lhsT = hprev[:, pcol + cc: pcol + cc + 1]
                    rhs = wh1_sb[:, (cc * 4 + g) * 512:(cc * 4 + g + 1) * 512]
                nc.tensor.matmul(ps[32 * g: 32 * g + 1, :], lhsT, rhs,
                                 start=(c == 0), stop=(c == 16),
                                 tile_position=(0, 32 * g))
        nc.vector.transpose(ts1[:], ps[:])
        strided = ts1[:].rearrange("p (a b) -> p a b", b=32)[:, :, 0:1]
        nc.scalar.activation(
            hk[:, i * 16:(i + 1) * 16].unsqueeze(-1), strided, Tanh)

    def l2_step(k, i):
        t = k * BT + i
        ps = qpool.tile([128, 512], F32, tag="pz2", name=f"pz2_{k}_{i}")
        if t < 2:
            nc.vector.memset(ps[:], 0.0)
        for c in range(17):
            for g in range(4):
                if c == 0:
                    lhsT = ident[:, i:i + 1]
                    rhs = a2[k % 2][:, g * 512:(g + 1) * 512]
                else:
                    cc = c - 1
                    lhsT = h2[:, (t % 2) * 16 + cc: (t % 2) * 16 + cc + 1]
                    rhs = wh2_sb[:, (cc * 4 + g) * 512:(cc * 4 + g + 1) * 512]
                nc.tensor.matmul(ps[32 * g: 32 * g + 1, :], lhsT, rhs,
                                 start=(c == 0), stop=(c == 16),
                                 tile_position=(0, 32 * g))
        nc.vector.transpose(ts2[:], ps[:])
        strided = ts2[:].rearrange("p (a b) -> p a b", b=32)[:, :, 0:1]
        nc.scalar.activation(
            h2[:, ((t + 1) % 2) * 16:((t + 1) % 2 + 1) * 16].unsqueeze(-1),
            strided, Tanh)

    # ---- prolog ----
    load_xt(0)
    batched_a1(0)
    load_xt(1)
    batched_a1(1)
    for i in range(BT):
        with tc.high_priority(offset=PRIO_OFF):
            l1_step(0, i)

    # ---- main pipeline ----
    for k in range(NB):
        batched_a2(k)
        if k + 2 < NB:
            load_xt(k + 2)
            batched_a1(k + 2)
        for i in range(BT):
            if k + 1 < NB:
                with tc.high_priority(offset=PRIO_OFF):
                    l1_step(k + 1, i)
            l2_step(k, i)

    # ---- epilog: out = h2_T @ W_h2o2.T + bo ----
    out_sb = sb("outsb", [1, OUT], F32)
    psos = [bpool.tile([128, 512], F32, tag="pb", name=f"pso{ns}")
            for ns in range(2)]
    for hh in range(2):  # contraction chunks 8*hh .. 8*hh+7
        wo = wscr.tile([128, 8 * 1024], BF16, tag="w", name=f"wo2_{hh}")
        nc.sync.dma_start(
            wo[:], ins["wo2t"].tensor.ap()[:, hh * 8 * OUT:(hh + 1) * 8 * OUT])
        for ns in range(2):
            ps = psos[ns][0:1, :]
            for kc in range(8):
                c = hh * 8 + kc
                nc.tensor.matmul(
                    ps, h2[:, (T % 2) * 16 + c: (T % 2) * 16 + c + 1],
                    wo[:, kc * OUT + ns * 512: kc * OUT + (ns + 1) * 512],
                    start=(c == 0), stop=False)
    for ns in range(2):
        ps = psos[ns][0:1, :]
        nc.tensor.matmul(ps, ones_row[:, 0:1], bo_sb[:, ns * 512:(ns + 1) * 512],
                         start=False, stop=True)
        nc.vector.tensor_copy(out_sb[:, ns * 512:(ns + 1) * 512], ps)
    nc.sync.dma_start(out_ap, out_sb[:])


_CACHE = {}


def _get_compiled():
    if "nc" in _CACHE:
        return _CACHE["nc"], _CACHE["in_names"]
    nc = bacc.Bacc("TRN2", target_bir_lowering=False, debug=False, num_devices=8)
    ins = {k: nc.dram_tensor(k, shp, dt, kind="ExternalInput")
           for k, (shp, dt) in _INPUT_SPECS.items()}
    out_dram = nc.dram_tensor("out", [1, OUT], F32, kind="ExternalOutput")
    with tile.TileContext(nc) as tc:
        with ExitStack() as ctx:
            _build(ctx, tc, out_dram.ap(), {k: v.ap() for k, v in ins.items()})
    nc.compile()
    _CACHE["nc"] = nc
    _CACHE["in_names"] = list(ins)
    return nc, list(ins)


def kernel(**inputs) -> np.ndarray:
    prep = _host_prep(inputs)
    nc, in_names = _get_compiled()
    in_map = {k: prep[k] for k in in_names}
    res = bass_utils.run_bass_kernel_spmd(
        nc, [in_map] * 8, core_ids=list(range(8)))
    return np.asarray(res.results[0]["out"], dtype=np.float32)


# revision 5
# speedup vs baseline: 1.0742x; 1.0742x over previous
"""Trainium2 Bass kernel for the 2-layer tanh RNN (nn_DeeperRNN), v2.

v2: interleave the two recurrences so the PE never idles.  T=512 is
processed in 4 blocks of 128 steps:

  prolog:  A1(b0), A1(b1) batched; l1-rec(b0)
  block k: A2(b k) batched + A1(b k+2) batched
           + l1-rec(b k+1) interleaved with l2-rec(b k)
  epilog:  out = h2_T @ W_h2o2.T + b_h2o2

The two recurrences have independent dependency chains, so the Tile
scheduler fills the PE gaps of one stream (its transpose+tanh latency)
with the other stream's gemv matmuls -> PE ~always busy.  Each per-step
gemv is 17 matmuls x 4 column-group streams (16 recurrent-weight chunks
+ 1 rank-1 inject of the batched pre-activation, which carries the
input term and both biases).  Only the two recurrent weight matrices
stay SBUF-resident; W_i2h1 / W_i2h2 / W_h2o2 stream through one 32KB
scratch slot.
"""

import sys
import numpy as np
import ml_dtypes

sys.path.insert(0, "/opt/trn_rl_repo")

import concourse.bass as bass  # noqa: E402
import concourse.mybir as mybir  # noqa: E402
import concourse.bacc as bacc  # noqa: E402
import concourse.tile as tile  # noqa: E402
import concourse.bass_utils as bass_utils  # noqa: E402
from contextlib import ExitStack  # noqa: E402

BF16 = mybir.dt.bfloat16
F32 = mybir.dt.float32
Tanh = mybir.ActivationFunctionType.Tanh

T, IN, H, OUT = 512, 1024, 2048, 1024
NCHUNK = H // 128  # 16
NB = 8             # blocks
BT = T // NB       # steps per block
PRIO_OFF = 1 << 20   # l1 chain outranks queued l2 work


def _host_prep(inputs):
    bf = ml_dtypes.bfloat16
    f32 = np.float32

    def perm_out_axis(a):
        # permute last axis: col (g, J, a2) = g*512 + 32*J + a2 <- row 128J + 32g + a2
        s = a.shape[:-1]
        return np.ascontiguousarray(
            a.reshape(*s, 16, 4, 32).swapaxes(-3, -2).reshape(*s, 2048)
        )

    def prep_wh(w):  # W [j, i] -> [128p, (c*4+g)*512 + J*32 + a2]
        wt = np.asarray(w, f32).T
        return np.ascontiguousarray(
            wt.reshape(16, 128, 16, 4, 32)
            .transpose(1, 0, 3, 2, 4)
            .reshape(128, 16 * 4 * 512)
            .astype(bf)
        )

    def pm(a, part=128):  # [K, N] -> [128, (K//128)*N] chunked partition-major
        k, n = a.shape
        return np.ascontiguousarray(
            a.reshape(k // part, part, n).transpose(1, 0, 2).reshape(part, -1)
        )

    x = np.asarray(inputs["word"], f32).reshape(T, IN)
    return {
        "xt": pm(np.ascontiguousarray(x.T).astype(bf)),
        "w1t": pm(perm_out_axis(np.asarray(inputs["W_i2h1"], f32).T).astype(bf)),
        "wi2t": pm(perm_out_axis(np.asarray(inputs["W_i2h2"], f32).T).astype(bf)),
        "wh1": prep_wh(inputs["W_h2h1"]),
        "wh2": prep_wh(inputs["W_h2h2"]),
        "wo2t": pm(np.asarray(inputs["W_h2o2"], f32).T.astype(bf)),
        "b1": perm_out_axis(
            np.asarray(inputs["b_i2h1"], f32) + np.asarray(inputs["b_h2h1"], f32)
        ).reshape(1, H).astype(bf),
        "b2": perm_out_axis(
            np.asarray(inputs["b_i2h2"], f32) + np.asarray(inputs["b_h2h2"], f32)
        ).reshape(1, H).astype(bf),
        "bo": np.asarray(inputs["b_h2o2"], f32).reshape(1, OUT).astype(bf),
        "ident": np.eye(128, dtype=bf),
        "ones_row": np.ones((1, 128), dtype=bf),
    }


_INPUT_SPECS = {
    "xt": ([128, (IN // 128) * T], BF16),
    "w1t": ([128, (IN // 128) * H], BF16),
    "wi2t": ([128, NCHUNK * H], BF16),
    "wh1": ([128, NCHUNK * 4 * 512], BF16),
    "wh2": ([128, NCHUNK * 4 * 512], BF16),
    "wo2t": ([128, NCHUNK * OUT], BF16),
    "b1": ([1, H], BF16),
    "b2": ([1, H], BF16),
    "bo": ([1, OUT], BF16),
    "ident": ([128, 128], BF16),
    "ones_row": ([1, 128], BF16),
}


def _build(ctx, tc, out_ap, ins):
    nc = tc.nc

    sb = lambda name, shape, dt: ctx.enter_context(nc.sbuf_tensor(name, shape, dt))

    ident = sb("identsb", [128, 128], BF16)
    nc.sync.dma_start(ident[:], ins["ident"])
    ones_row = sb("onessb", [1, 128], BF16)
    nc.sync.dma_start(ones_row[:], ins["ones_row"])
    b1_sb = sb("b1sb", [1, H], BF16)
    nc.sync.dma_start(b1_sb[:], ins["b1"])
    b2_sb = sb("b2sb", [1, H], BF16)
    nc.sync.dma_start(b2_sb[:], ins["b2"])
    bo_sb = sb("bosb", [1, OUT], BF16)
    nc.sync.dma_start(bo_sb[:], ins["bo"])

    # resident recurrent weights (64KB/partition each)
    wh1_sb = sb("wh1sb", [128, NCHUNK * 4 * 512], BF16)
    nc.sync.dma_start(wh1_sb[:], ins["wh1"])
    wh2_sb = sb("wh2sb", [128, NCHUNK * 4 * 512], BF16)
    nc.sync.dma_start(wh2_sb[:], ins["wh2"])

    # per-block pre-activations [128 t-part, 2048] bf16, rings of 2
    # (rows BT..127 stay zero; the ident-column inject must see 0 there)
    a1 = [sb(f"a1_{i}", [128, H], BF16) for i in range(2)]
    a2 = [sb(f"a2_{i}", [128, H], BF16) for i in range(2)]
    for t_ in a1 + a2:
        nc.vector.memset(t_[:], 0.0)

    # h1: ring of 3 blocks, col (i*16 + c) = h1[step i][chunk c]
    h1 = [sb(f"h1_{i}", [128, BT * 16], BF16) for i in range(3)]
    h1z = sb("h1z", [128, 16], BF16)  # h1[-1] = 0
    nc.vector.memset(h1z[:], 0.0)
    # h2: ring of 2 slots (only previous step needed) + epilog reads slot T%2
    h2 = sb("h2sb", [128, 2 * 16], BF16)
    nc.vector.memset(h2[:, 0:16], 0.0)

    ts1 = sb("ts1", [128, 512], F32)  # transpose scratch, l1 stream
    ts2 = sb("ts2", [128, 512], F32)  # transpose scratch, l2 stream

    # x blocks [128, 8*BT] bf16, ring 2 (chunk-major: col kc*BT + i)
    xt_blk = [sb(f"xtb_{i}", [128, (IN // 128) * BT], BF16) for i in range(2)]

    # streamed-weight scratch: one 32KB slot (16K bf16 cols)
    wscr = ctx.enter_context(tc.tile_pool(name="wscr", bufs=1))

    ppool = ctx.enter_context(tc.tile_pool(name="ppool", bufs=2, space="PSUM"))
    qpool = ctx.enter_context(tc.tile_pool(name="qpool", bufs=2, space="PSUM"))
    bpool = ctx.enter_context(tc.tile_pool(name="bpool", bufs=4, space="PSUM"))

    def load_xt(k):
        dst = xt_blk[k % 2]
        for kc in range(IN // 128):
            nc.sync.dma_start(
                dst[:, kc * BT:(kc + 1) * BT],
                ins["xt"].tensor.ap()[:, kc * T + k * BT: kc * T + k * BT + BT])

    def batched_a1(k):
        """A1 block k: [128 t, 2048 j] = X_blk @ W1.T + (b1+bh1)."""
        xk = xt_blk[k % 2]
        adst = a1[k % 2]
        pss = [bpool.tile([128, 512], F32, tag="pb", name=f"pa1_{k}_{ns}")
               for ns in range(4)]
        for half in range(2):  # contraction chunks 4*half .. 4*half+3
            w1 = wscr.tile([128, 8 * 1024], BF16, tag="w", name=f"w1t_{k}_{half}")
            nc.sync.dma_start(
                w1[:], ins["w1t"].tensor.ap()[:, half * 4 * H:(half + 1) * 4 * H])
            for ns in range(4):
                for kc in range(4):
                    c = half * 4 + kc
                    nc.tensor.matmul(
                        pss[ns][0:BT, :], xk[:, c * BT:(c + 1) * BT],
                        w1[:, kc * H + ns * 512: kc * H + (ns + 1) * 512],
                        start=(c == 0), stop=False)
        for ns in range(4):
            nc.tensor.matmul(
                pss[ns][0:BT, :], ones_row[:, 0:BT], b1_sb[:, ns * 512:(ns + 1) * 512],
                start=False, stop=True)
            nc.vector.tensor_copy(adst[0:BT, ns * 512:(ns + 1) * 512], pss[ns][0:BT, :])

    def batched_a2(k):
        """A2 block k = H1(block k) @ Wi2.T + (b2+bh2); wi2t streamed in
        2 contraction halves through the scratch slot."""
        hv = h1[k % 3][:].rearrange("p (t c) -> p t c", c=16)
        adst = a2[k % 2]
        pss = [bpool.tile([128, 512], F32, tag="pb", name=f"pa2_{k}_{ns}")
               for ns in range(4)]
        for q in range(4):
            wq = wscr.tile([128, 8 * 1024], BF16, tag="w", name=f"wi2_{k}_{q}")
            nc.sync.dma_start(
                wq[:], ins["wi2t"].tensor.ap()[:, q * 4 * H:(q + 1) * 4 * H])
            for ns in range(4):
                for kc in range(4):
                    c = q * 4 + kc
                    nc.tensor.matmul(
                        pss[ns][0:BT, :], hv[:, 0:BT, c:c + 1],
                        wq[:, kc * H + ns * 512: kc * H + (ns + 1) * 512],
                        start=(c == 0), stop=False)
        for ns in range(4):
            nc.tensor.matmul(
                pss[ns][0:BT, :], ones_row[:, 0:BT], b2_sb[:, ns * 512:(ns + 1) * 512],
                start=False, stop=True)
            nc.vector.tensor_copy(adst[0:BT, ns * 512:(ns + 1) * 512], pss[ns][0:BT, :])

    def l1_step(k, i):
        hk = h1[k % 3]
        if i == 0:
            hprev, pcol = (h1z, 0) if k == 0 else (h1[(k - 1) % 3], (BT - 1) * 16)
        else:
            hprev, pcol = hk, (i - 1) * 16
        ps = ppool.tile([128, 512], F32, tag="pz", name=f"pz1_{k}_{i}")
        if k == 0 and i < 2:
            nc.vector.memset(ps[:], 0.0)
        for c in range(17):
            for g in range(4):
                if c == 0:
                    lhsT = ident[:, i:i + 1]
                    rhs = a1[k % 2][:, g * 512:(g + 1) * 512]
                else:
                    cc = c - 1
                    lhsT = hprev[:, pcol + cc: pcol + cc + 1]
                    rhs = wh1_sb[:, (cc * 4 + g) * 512:(cc * 4 + g + 1) * 512]
                nc.tensor.matmul(ps[32 * g: 32 * g + 1, :], lhsT, rhs,
                                 start=(c == 0), stop=(c == 16),
                                 tile_position=(0, 32 * g))
        nc.vector.transpose(ts1[:], ps[:])
        strided = ts1[:].rearrange("p (a b) -> p a b", b=32)[:, :, 0:1]
        nc.scalar.activation(
            hk[:, i * 16:(i + 1) * 16].unsqueeze(-1), strided, Tanh)

    def l2_step(k, i):
        t = k * BT + i
        ps = qpool.tile([128, 512], F32, tag="pz2", name=f"pz2_{k}_{i}")
        if t < 2:
            nc.vector.memset(ps[:], 0.0)
        for c in range(17):
            for g in range(4):
                if c == 0:
                    lhsT = ident[:, i:i + 1]
                    rhs = a2[k % 2][:, g * 512:(g + 1) * 512]
                else:
                    cc = c - 1
                    lhsT = h2[:, (t % 2) * 16 + cc: (t % 2) * 16 + cc + 1]
                    rhs = wh2_sb[:, (cc * 4 + g) * 512:(cc * 4 + g + 1) * 512]
                nc.tensor.matmul(ps[32 * g: 32 * g + 1, :], lhsT, rhs,
                                 start=(c == 0), stop=(c == 16),
                                 tile_position=(0, 32 * g))
        nc.vector.transpose(ts2[:], ps[:])
        strided = ts2[:].rearrange("p (a b) -> p a b", b=32)[:, :, 0:1]
        nc.scalar.activation(
            h2[:, ((t + 1) % 2) * 16:((t + 1) % 2 + 1) * 16].unsqueeze(-1),
            strided, Tanh)

    # ---- prolog ----
    load_xt(0)
    batched_a1(0)
    load_xt(1)
    batched_a1(1)
    for i in range(BT):
        with tc.high_priority(offset=PRIO_OFF):
            l1_step(0, i)

    # ---- main pipeline ----
    for k in range(NB):
        batched_a2(k)
        if k + 2 < NB:
            load_xt(k + 2)
            batched_a1(k + 2)
        for i in range(BT):
            if k + 1 < NB:
                with tc.high_priority(offset=PRIO_OFF):
                    l1_step(k + 1, i)
            l2_step(k, i)

    # ---- epilog: out = h2_T @ W_h2o2.T + bo ----
    out_sb = sb("outsb", [1, OUT], F32)
    psos = [bpool.tile([128, 512], F32, tag="pb", name=f"pso{ns}")
            for ns in range(2)]
    for hh in range(2):  # contraction chunks 8*hh .. 8*hh+7
        wo = wscr.tile([128, 8 * 1024], BF16, tag="w", name=f"wo2_{hh}")
        nc.sync.dma_start(
            wo[:], ins["wo2t"].tensor.ap()[:, hh * 8 * OUT:(hh + 1) * 8 * OUT])
        for ns in range(2):
            ps = psos[ns][0:1, :]
            for kc in range(8):
                c = hh * 8 + kc
                nc.tensor.matmul(
                    ps, h2[:, (T % 2) * 16 + c: (T % 2) * 16 + c + 1],
                    wo[:, kc * OUT + ns * 512: kc * OUT + (ns + 1) * 512],
                    start=(c == 0), stop=False)
    for ns in range(2):
        ps = psos[ns][0:1, :]
        nc.tensor.matmul(ps, ones_row[:, 0:1], bo_sb[:, ns * 512:(ns + 1) * 512],
                         start=False, stop=True)
        nc.vector.tensor_copy(out_sb[:, ns * 512:(ns + 1) * 512], ps)
    nc.sync.dma_start(out_ap, out_sb[:])


_CACHE = {}


def _get_compiled():
    if "nc" in _CACHE:
        return _CACHE["nc"], _CACHE["in_names"]
    nc = bacc.Bacc("TRN2", target_bir_lowering=False, debug=False, num_devices=8)
    ins = {k: nc.dram_tensor(k, shp, dt, kind="ExternalInput")
           for k, (shp, dt) in _INPUT_SPECS.items()}
    out_dram = nc.dram_tensor("out", [1, OUT], F32, kind="ExternalOutput")
    with tile.TileContext(nc) as tc:
        with ExitStack() as ctx:
            _build(ctx, tc, out_dram.ap(), {k: v.ap() for k, v in ins.items()})
    nc.compile()
    _CACHE["nc"] = nc
    _CACHE["in_names"] = list(ins)
    return nc, list(ins)


def kernel(**inputs) -> np.ndarray:
    prep = _host_prep(inputs)
    nc, in_names = _get_compiled()
    in_map = {k: prep[k] for k in in_names}
    res = bass_utils.run_bass_kernel_spmd(
        nc, [in_map] * 8, core_ids=list(range(8)))
    return np.asarray(res.results[0]["out"], dtype=np.float32)
